# revision 21
# baseline (speedup 1.0000x reference)
"""Causal self-attention (B=2, T=2048, C=1024, H=16, D=64) on 8 TRN2 cores.

Sharding: core = b*4 + g handles batch b, heads 4g..4g+3 (data parallel on B,
tensor parallel on heads). Each core computes its 4 heads' contribution to
x @ W_proj; host sums the 4 partial outputs per batch and adds b_proj.

v2 redesign vs the 341us baseline (which was PE-cold/ACT-thrash bound):
  - All inputs host-pre-laid-out to SBUF layout -> 9 big contiguous DMAs
    (baseline: 74 small DMAs x ~650ns serial issue = 50us dead prologue).
  - Single software pipeline over tci: A(tci) qkv-proj + per-chunk RoPE,
    then B(p, qc=tci) flash attention, then C(qc=tci) out-proj, so the
    Tile scheduler can overlap A(tci+1) PE work under B(tci) ACT exp and
    keep the PE HAM-warm throughout.
  - exp/ln pinned to the natural_log_exp_and_others ACT table set via a
    get_activation_tables monkeypatch (baseline thrashed 19 table loads).
  - RoPE swap-halves fused into the sin-multiply with partition-block
    strided APs (2 TTs), PSUM evacuations on DVE, softmax denominators
    ln/exp'd on ACT straight out of PSUM, one broadcast matmul per (p,qc).
  - bf16 everywhere off-chip except nothing: out is bf16, host accumulates
    in fp32.
"""
import os
import numpy as np

import concourse.bass as bass
import concourse.mybir as mybir
from concourse import bacc
from concourse.tile import TileContext
from concourse.bass_utils import run_bass_kernel_spmd

B, T, C, H, D = 2, 2048, 1024, 16, 64
HPC = 4          # heads per core
NCORES = 8
TCH = 512        # t-chunk / q-chunk width
NTC = T // TCH   # 4
NTT = T // 128   # 16 t-tiles
NCC = C // 128   # 8 c-chunks
F32 = mybir.dt.float32
BF16 = mybir.dt.bfloat16
MMDT = BF16
AF = mybir.ActivationFunctionType
ALU = mybir.AluOpType

_prog_cache = {}
_DEBUG = bool(os.environ.get("TRNK_DEBUG"))


def _patch_act_tables():
    """Make every exp/ln activation resolve to natural_log_exp_and_others
    so the kernel needs exactly one ACT table load (the baseline thrashed
    19 loads between exp_and_others and natural_log)."""
    try:
        import concourse.hw_specs as _hw
        import concourse.bacc as _bc
        orig = _hw.get_activation_tables
        if getattr(orig, "_trnk_patched", False):
            return

        def patched(arch):
            tabs = orig(arch)
            keep = "natural_log_exp_and_others"
            if keep in tabs:
                for name, s in tabs.items():
                    if name != keep:
                        s.discard(AF.Exp)
                        s.discard(AF.Ln)
            return tabs

        patched._trnk_patched = True
        _hw.get_activation_tables = patched
        _bc.get_activation_tables = patched
        try:
            import concourse.bass_interp as _bi
            _bi.get_activation_tables = patched
        except Exception:
            pass
    except Exception:
        pass


def _blk(ap):
    """View a [128, N] AP as partition blocks of 32: [2, 2, 32, N]
    (index [a, b]: partitions a*64 + b*32 + 0..31)."""
    return ap.rearrange("(a b s) t -> a b s t", a=2, b=2, s=32)


def _build_program(has_battn: bool):
    nc = bacc.Bacc("TRN2", target_bir_lowering=False, debug=False,
                   num_devices=NCORES)
    # ---- DRAM I/O (per core, all pre-laid-out to SBUF layout) ----
    xt_d = nc.dram_tensor("xt", [128, NTC * NCC * TCH], MMDT, kind="ExternalInput")
    wqk_d = nc.dram_tensor("wqk", [128, NCC * 512], MMDT, kind="ExternalInput")
    wv_d = nc.dram_tensor("wv", [128, NCC * HPC * D], MMDT, kind="ExternalInput")
    wp_d = nc.dram_tensor("wp", [128, 2 * C], MMDT, kind="ExternalInput")
    trig_d = nc.dram_tensor("trig", [128, 2 * T], MMDT, kind="ExternalInput")
    mask_d = nc.dram_tensor("masks", [128, 4 * TCH], MMDT, kind="ExternalInput")
    hsel_d = nc.dram_tensor("hsel", [2, 128], MMDT, kind="ExternalInput")
    out_d = nc.dram_tensor("out", [T, C], MMDT, kind="ExternalOutput")
    if _DEBUG:
        dbg_q0 = nc.dram_tensor("dbg_q0", [128, T], F32, kind="ExternalOutput")
        dbg_k0 = nc.dram_tensor("dbg_k0", [128, T], F32, kind="ExternalOutput")
        dbg_y0 = nc.dram_tensor("dbg_y0", [128, T], F32, kind="ExternalOutput")
        dbg_va = nc.dram_tensor("dbg_va", [128, HPC * (D + 1)], F32, kind="ExternalOutput")
        dbg_et = nc.dram_tensor("dbg_et", [128, 2 * TCH], F32, kind="ExternalOutput")
        dbg_dr = nc.dram_tensor("dbg_dr", [1, 2 * TCH], F32, kind="ExternalOutput")
    if has_battn:
        bqk_d = nc.dram_tensor("bqk", [128, 4], F32, kind="ExternalInput")
        vbias_d = nc.dram_tensor("vbias", [128, HPC * D], MMDT, kind="ExternalInput")

    with TileContext(nc) as tc:
        with (
            tc.tile_pool(name="wsb", bufs=1) as wsb,      # persistent weights/tables
            tc.tile_pool(name="xsb", bufs=4) as xsb,      # streamed xt chunks
            tc.tile_pool(name="qk", bufs=1) as qksb,      # persistent qT/kT/yT/v
            tc.tile_pool(name="rsb", bufs=4) as rsb,      # rope swap staging
            tc.tile_pool(name="esb", bufs=6) as esb,      # exp tiles
            tc.tile_pool(name="nsb", bufs=2) as nsb,      # norm small tiles
            tc.tile_pool(name="osb", bufs=3) as osb,      # out staging
            tc.tile_pool(name="ps", bufs=2, space="PSUM") as ps,
        ):
            # ---- persistent loads (few big DMAs) ----
            wqk_sb = wsb.tile([128, NCC * 512], MMDT, tag="wqk")
            nc.sync.dma_start(out=wqk_sb[:], in_=wqk_d[:])
            wv_sb = wsb.tile([128, NCC * HPC * D], MMDT, tag="wv")
            nc.sync.dma_start(out=wv_sb[:], in_=wv_d[:])
            trig_sb = wsb.tile([128, 2 * T], MMDT, tag="trig")
            nc.sync.dma_start(out=trig_sb[:], in_=trig_d[:])
            mask_sb = wsb.tile([128, 4 * TCH], MMDT, tag="masks")
            nc.sync.dma_start(out=mask_sb[:], in_=mask_d[:])
            ones_sb = wsb.tile([1, 64], MMDT, tag="ones1")
            nc.sync.dma_start(out=ones_sb[:], in_=hsel_d[0:1, 0:64])
            wp_sb = wsb.tile([128, 2 * C], MMDT, tag="wp")
            nc.sync.dma_start(out=wp_sb[:], in_=wp_d[:])
            if has_battn:
                bqk_sb = wsb.tile([128, 4], F32, tag="bqk")
                nc.sync.dma_start(out=bqk_sb[:], in_=bqk_d[:])
                vbias_sb = wsb.tile([128, HPC * D], MMDT, tag="vbias")
                nc.sync.dma_start(out=vbias_sb[:], in_=vbias_d[:])

            # persistent activations
            qT = [qksb.tile([128, T], MMDT, tag=f"qT{p}", name=f"qT{p}") for p in range(2)]
            kT = [qksb.tile([128, T], MMDT, tag=f"kT{p}", name=f"kT{p}") for p in range(2)]
            yT = [qksb.tile([128, T], MMDT, tag=f"yT{p}", name=f"yT{p}") for p in range(2)]
            vaug = [qksb.tile([128, HPC * (D + 1)], MMDT, tag=f"va{tt}", name=f"va{tt}")
                    for tt in range(NTT)]
            # ones columns of v_aug via on-chip memset (cols h*(D+1)+D)
            for tt in range(NTT):
                nc.vector.memset(vaug[tt][:, D::D + 1], 1.0)

            qk_dst = [qT[0], qT[1], kT[0], kT[1]]
            # prefetch all x chunks up front
            xts = []
            for tci in range(NTC):
                xt = xsb.tile([128, NCC * TCH], MMDT, tag="xt", name=f"xt{tci}")
                nc.sync.dma_start(
                    out=xt[:], in_=xt_d[:, tci * NCC * TCH:(tci + 1) * NCC * TCH])
                xts.append(xt)
            for tci in range(NTC):
                # ---- Phase A: qkv projection + RoPE for t-chunk tci ----
                xt = xts[tci]
                cw = slice(tci * TCH, (tci + 1) * TCH)
                cosc = trig_sb[:, cw]
                sinc = trig_sb[:, T + tci * TCH: T + (tci + 1) * TCH]
                for ft in (2, 0, 3, 1):  # k_p0, q_p0, k_p1, q_p1
                    pqk = ps.tile([128, TCH], F32, tag="pa", name=f"pqk_{tci}_{ft}")
                    for cc in range(NCC):
                        nc.tensor.matmul(
                            pqk[:],
                            wqk_sb[:, cc * 512 + ft * 128: cc * 512 + (ft + 1) * 128],
                            xt[:, cc * TCH:(cc + 1) * TCH],
                            start=(cc == 0), stop=(cc == NCC - 1))
                    Xc = qk_dst[ft][:, cw]
                    if has_battn:
                        nc.scalar.activation(Xc, pqk[:], AF.Identity,
                                             bias=bqk_sb[:, ft:ft + 1])
                    else:
                        nc.vector.tensor_copy(Xc, pqk[:])
                    # RoPE immediately (swap halves via SBUF->SBUF DMAs)
                    X = qk_dst[ft]
                    xs = rsb.tile([128, TCH], MMDT, tag="xswap", name=f"xs_{tci}_{ft}")
                    nc.gpsimd.dma_start(out=xs[0:32, :], in_=X[32:64, cw])
                    nc.gpsimd.dma_start(out=xs[32:64, :], in_=X[0:32, cw])
                    nc.gpsimd.dma_start(out=xs[64:96, :], in_=X[96:128, cw])
                    nc.sync.dma_start(out=xs[96:128, :], in_=X[64:96, cw])
                    nc.vector.tensor_tensor(xs[:], xs[:], sinc, ALU.mult)
                    nc.vector.tensor_tensor(Xc, Xc, cosc, ALU.mult)
                    nc.vector.tensor_tensor(Xc, Xc, xs[:], ALU.add)
                # v: two j-pairs, each one PSUM bank
                for jj in range(2):
                    pv = ps.tile([128, 2 * HPC * D], F32, tag="pa", name=f"pv_{tci}_{jj}")
                    for j2 in range(2):
                        j = jj * 2 + j2
                        for cc in range(NCC):
                            nc.tensor.matmul(
                                pv[:, j2 * 256:(j2 + 1) * 256],
                                xt[:, cc * TCH + j * 128: cc * TCH + (j + 1) * 128],
                                wv_sb[:, cc * 256:(cc + 1) * 256],
                                start=(cc == 0), stop=(cc == NCC - 1))
                    for j2 in range(2):
                        tt = tci * 4 + jj * 2 + j2
                        dst = vaug[tt][:, 0:HPC * (D + 1)].rearrange(
                            "p (h e) -> p h e", e=D + 1)[:, :, 0:D]
                        src = pv[:, j2 * 256:(j2 + 1) * 256].rearrange(
                            "p (h e) -> p h e", e=D)
                        if has_battn:
                            nc.vector.scalar_tensor_tensor(
                                dst, src, 0.0,
                                vbias_sb[:].rearrange("p (h e) -> p h e", e=D),
                                ALU.add, ALU.add)
                        else:
                            nc.vector.tensor_copy(dst, src)

                # ---- Phase B: causal attention for q-chunk qc = tci ----
                qc = tci
                nk = 4 * qc + 4
                for p in range(2):
                    yps = [ps.tile([D + 1, TCH], F32, tag="yacc",
                                   name=f"yps_{p}_{qc}_{h}") for h in range(2)]
                    for ktp in range(nk // 2):
                        sc = [ps.tile([128, 2 * TCH], F32, tag="sc",
                                      name=f"sc_{p}_{qc}_{ktp}_{h}") for h in range(2)]
                        et = [esb.tile([128, 2 * TCH], MMDT, tag="et",
                                       name=f"et_{p}_{qc}_{ktp}_{h}") for h in range(2)]
                        for half in range(2):
                            kt = 2 * ktp + half
                            for h in range(2):
                                nc.tensor.matmul(
                                    sc[h][:, half * TCH:(half + 1) * TCH],
                                    kT[p][h * 64:(h + 1) * 64,
                                          kt * 128:(kt + 1) * 128],
                                    qT[p][h * 64:(h + 1) * 64, cw],
                                    start=True, stop=True,
                                    tile_position=(64 * h, 0))
                        for h in range(2):
                            nc.scalar.activation(et[h][:], sc[h][:], AF.Exp,
                                                 scale=0.125)
                        if _DEBUG and p == 0 and qc == 0 and ktp == 0:
                            dt_ = osb.tile([128, 2 * TCH], F32, tag="dbge")
                            nc.vector.tensor_copy(dt_[:], et[0][:])
                            nc.sync.dma_start(out=dbg_et[:], in_=dt_[:])
                        # causal masking for diagonal-crossing tiles
                        for half in range(2):
                            kt = 2 * ktp + half
                            m = kt - 4 * qc
                            if m >= 0:
                                w = 128 * (m + 1)
                                off = half * TCH
                                for h in range(2):
                                    nc.vector.tensor_tensor(
                                        et[h][:, off:off + w],
                                        et[h][:, off:off + w],
                                        mask_sb[:, m * TCH: m * TCH + w],
                                        ALU.mult)
                        # attn @ v
                        for half in range(2):
                            kt = 2 * ktp + half
                            for h in range(2):
                                hh = 2 * p + h
                                nc.tensor.matmul(
                                    yps[h][:],
                                    vaug[kt][:, hh * (D + 1):(hh + 1) * (D + 1)],
                                    et[h][:, half * TCH:(half + 1) * TCH],
                                    start=(kt == 0), stop=(kt == nk - 1))
                    # normalize: recip = exp(-ln(den)); broadcast via 2 matmuls
                    dln = nsb.tile([1, 2 * TCH], F32, tag="dln", name=f"dln_{p}_{qc}")
                    for h in range(2):
                        nc.scalar.activation(dln[0:1, h * TCH:(h + 1) * TCH],
                                             yps[h][D:D + 1, :], AF.Ln)
                    drec = nsb.tile([1, 2 * TCH], MMDT, tag="drec", name=f"drec_{p}_{qc}")
                    nc.scalar.activation(drec[:], dln[:], AF.Exp, scale=-1.0)
                    pb = ps.tile([128, TCH], F32, tag="pa", name=f"pb_{p}_{qc}")
                    for h in range(2):
                        nc.tensor.matmul(pb[h * 64:(h + 1) * 64, :], ones_sb[:],
                                         drec[0:1, h * TCH:(h + 1) * TCH],
                                         start=True, stop=True,
                                         tile_position=(0, 64 * h))
                    rb = nsb.tile([128, TCH], MMDT, tag="rb", name=f"rb_{p}_{qc}")
                    nc.vector.tensor_copy(rb[:], pb[:])
                    if _DEBUG and p == 0 and qc == 0:
                        dr_ = osb.tile([1, 2 * TCH], F32, tag="dbgr")
                        nc.vector.tensor_copy(dr_[:], dln[:])
                        nc.sync.dma_start(out=dbg_dr[:], in_=dr_[:])
                    for h in range(2):
                        nc.vector.tensor_tensor(
                            yT[p][h * 64:(h + 1) * 64, cw],
                            yps[h][0:D, :], rb[h * 64:(h + 1) * 64, :], ALU.mult)

                # ---- Phase C: output projection for q-chunk qc ----
                for tt in range(4 * qc, 4 * qc + 4):
                    for nchunk in range(2):
                        pp = ps.tile([128, TCH], F32, tag="pa",
                                     name=f"pp_{tt}_{nchunk}")
                        for kk in range(2):
                            nc.tensor.matmul(
                                pp[:],
                                yT[kk][:, tt * 128:(tt + 1) * 128],
                                wp_sb[:, kk * C + nchunk * TCH:
                                      kk * C + (nchunk + 1) * TCH],
                                start=(kk == 0), stop=(kk == 1))
                        ot = osb.tile([128, TCH], MMDT, tag="ot",
                                      name=f"ot_{tt}_{nchunk}")
                        if qc == NTC - 1:
                            nc.scalar.copy(ot[:], pp[:])
                        else:
                            nc.vector.tensor_copy(ot[:], pp[:])
                        nc.sync.dma_start(
                            out=out_d[tt * 128:(tt + 1) * 128,
                                      nchunk * TCH:(nchunk + 1) * TCH],
                            in_=ot[:])

            if _DEBUG:
                for src_t, dst_t in [(qT[0], dbg_q0), (kT[0], dbg_k0),
                                     (yT[0], dbg_y0)]:
                    dt_ = osb.tile([128, T], F32, tag="dbg")
                    nc.vector.tensor_copy(dt_[:], src_t[:])
                    nc.sync.dma_start(out=dst_t[:], in_=dt_[:])
                dv_ = osb.tile([128, HPC * (D + 1)], F32, tag="dbgv")
                nc.vector.tensor_copy(dv_[:], vaug[0][:])
                nc.sync.dma_start(out=dbg_va[:], in_=dv_[:])

    nc.finalize()
    return nc


def _rope_tables():
    dd = (np.arange(128) % 64) % 32
    fraction = (2.0 * np.arange(32, dtype=np.float32) / 64).astype(np.float32)
    timescale = (np.float32(10000.0) ** fraction).astype(np.float32)
    pos = np.arange(T, dtype=np.float32)
    ang = (pos[None, :] / timescale[dd][:, None]).astype(np.float32)  # [128, T]
    cos_t = np.cos(ang).astype(np.float32)
    sin_t = np.sin(ang).astype(np.float32)
    sgn = np.where((np.arange(128) % 64) < 32, np.float32(-1.0), np.float32(1.0))
    sin_signed = (sin_t * sgn[:, None]).astype(np.float32)
    return cos_t, sin_signed


def _mask_tiles():
    masks = np.zeros((128, 4 * TCH), np.float32)
    r = np.arange(128)[:, None]
    c = np.arange(TCH)[None, :]
    for m in range(4):
        masks[:, m * TCH:(m + 1) * TCH] = (c >= 128 * m + r).astype(np.float32)
    return masks


def kernel(x, W_attn, b_attn, W_proj, b_proj):
    x = np.asarray(x, np.float32)
    W_attn = np.asarray(W_attn, np.float32)
    b_attn = np.asarray(b_attn, np.float32)
    W_proj = np.asarray(W_proj, np.float32)
    b_proj = np.asarray(b_proj, np.float32)

    _patch_act_tables()
    has_battn = bool(np.any(b_attn != 0))
    key = ("v5", has_battn, _DEBUG)
    if key not in _prog_cache:
        _prog_cache[key] = _build_program(has_battn)
    nc = _prog_cache[key]

    import ml_dtypes
    bf = ml_dtypes.bfloat16
    cos_t, sin_signed = _rope_tables()
    trig = np.concatenate([cos_t, sin_signed], axis=1).astype(bf)  # [128, 4096]
    masks = _mask_tiles().astype(bf)
    hsel = np.ones((2, 128), bf)

    in_maps = []
    for core in range(NCORES):
        b, g = divmod(core, HPC)
        hs = [HPC * g + i for i in range(HPC)]
        qkcols = []
        for base in (0, C):  # q tiles then k tiles
            for p in range(2):
                for i in (2 * p, 2 * p + 1):
                    qkcols += [base + hs[i] * D + d for d in range(D)]
        vcols = [2 * C + h * D + d for h in hs for d in range(D)]
        rows = [h * D + d for h in hs for d in range(D)]

        wqk = np.ascontiguousarray(
            W_attn[:, qkcols].reshape(NCC, 128, 512).transpose(1, 0, 2)
            .reshape(128, NCC * 512)).astype(bf)
        wv = np.ascontiguousarray(
            W_attn[:, vcols].reshape(NCC, 128, 256).transpose(1, 0, 2)
            .reshape(128, NCC * 256)).astype(bf)
        wp = np.ascontiguousarray(
            W_proj[rows, :].reshape(2, 128, C).transpose(1, 0, 2)
            .reshape(128, 2 * C)).astype(bf)
        xt = np.ascontiguousarray(
            x[b].reshape(NTC, TCH, NCC, 128).transpose(3, 0, 2, 1)
            .reshape(128, NTC * NCC * TCH)).astype(bf)

        im = {
            "xt": xt, "wqk": wqk, "wv": wv, "wp": wp,
            "trig": trig, "masks": masks, "hsel": hsel,
        }
        if has_battn:
            im["bqk"] = np.ascontiguousarray(
                b_attn[qkcols].reshape(4, 128).T).astype(np.float32)
            im["vbias"] = np.tile(b_attn[vcols], (128, 1)).astype(bf)
        in_maps.append(im)

    trace = bool(os.environ.get("TRNK_TRACE"))
    if trace:
        try:
            import ntff_shim  # noqa: F401
        except ImportError:
            trace = False
    res = run_bass_kernel_spmd(nc, in_maps, list(range(NCORES)), trace=trace,
                               tmpdir=os.environ.get("TRNK_TMPDIR") or None)
    if trace:
        globals()["_last_exec_time_ns"] = res.exec_time_ns
        globals()["_last_result"] = res

    globals()["_dbg_results"] = res.results
    out = np.zeros((B, T, C), np.float32)
    for core in range(NCORES):
        b = core // HPC
        out[b] += np.asarray(res.results[core]["out"], np.float32)
    out += b_proj[None, None, :]
    return out


# revision 23
# speedup vs baseline: 1.0050x; 1.0050x over previous
"""Causal self-attention (B=2, T=2048, C=1024, H=16, D=64) on 8 TRN2 cores.

Sharding: core = b*4 + g handles batch b, heads 4g..4g+3 (data parallel on B,
tensor parallel on heads). Each core computes its 4 heads' contribution to
x @ W_proj; host sums the 4 partial outputs per batch and adds b_proj.

v2 redesign vs the 341us baseline (which was PE-cold/ACT-thrash bound):
  - All inputs host-pre-laid-out to SBUF layout -> 9 big contiguous DMAs
    (baseline: 74 small DMAs x ~650ns serial issue = 50us dead prologue).
  - Single software pipeline over tci: A(tci) qkv-proj + per-chunk RoPE,
    then B(p, qc=tci) flash attention, then C(qc=tci) out-proj, so the
    Tile scheduler can overlap A(tci+1) PE work under B(tci) ACT exp and
    keep the PE HAM-warm throughout.
  - exp/ln pinned to the natural_log_exp_and_others ACT table set via a
    get_activation_tables monkeypatch (baseline thrashed 19 table loads).
  - RoPE swap-halves fused into the sin-multiply with partition-block
    strided APs (2 TTs), PSUM evacuations on DVE, softmax denominators
    ln/exp'd on ACT straight out of PSUM, one broadcast matmul per (p,qc).
  - bf16 everywhere off-chip except nothing: out is bf16, host accumulates
    in fp32.
"""
import os
import numpy as np

import concourse.bass as bass
import concourse.mybir as mybir
from concourse import bacc
from concourse.tile import TileContext
from concourse.bass_utils import run_bass_kernel_spmd

B, T, C, H, D = 2, 2048, 1024, 16, 64
HPC = 4          # heads per core
NCORES = 8
TCH = 512        # t-chunk / q-chunk width
NTC = T // TCH   # 4
NTT = T // 128   # 16 t-tiles
NCC = C // 128   # 8 c-chunks
F32 = mybir.dt.float32
BF16 = mybir.dt.bfloat16
MMDT = BF16
AF = mybir.ActivationFunctionType
ALU = mybir.AluOpType

_prog_cache = {}
_DEBUG = bool(os.environ.get("TRNK_DEBUG"))


def _patch_act_tables():
    """Make every exp/ln activation resolve to natural_log_exp_and_others
    so the kernel needs exactly one ACT table load (the baseline thrashed
    19 loads between exp_and_others and natural_log)."""
    try:
        import concourse.hw_specs as _hw
        import concourse.bacc as _bc
        orig = _hw.get_activation_tables
        if getattr(orig, "_trnk_patched", False):
            return

        def patched(arch):
            tabs = orig(arch)
            keep = "natural_log_exp_and_others"
            if keep in tabs:
                for name, s in tabs.items():
                    if name != keep:
                        s.discard(AF.Exp)
                        s.discard(AF.Ln)
            return tabs

        patched._trnk_patched = True
        _hw.get_activation_tables = patched
        _bc.get_activation_tables = patched
        try:
            import concourse.bass_interp as _bi
            _bi.get_activation_tables = patched
        except Exception:
            pass
    except Exception:
        pass


def _blk(ap):
    """View a [128, N] AP as partition blocks of 32: [2, 2, 32, N]
    (index [a, b]: partitions a*64 + b*32 + 0..31)."""
    return ap.rearrange("(a b s) t -> a b s t", a=2, b=2, s=32)


def _build_program(has_battn: bool):
    nc = bacc.Bacc("TRN2", target_bir_lowering=False, debug=False,
                   num_devices=NCORES)
    # ---- DRAM I/O (per core, all pre-laid-out to SBUF layout) ----
    xt_d = nc.dram_tensor("xt", [128, NTC * NCC * TCH], MMDT, kind="ExternalInput")
    wqk_d = nc.dram_tensor("wqk", [128, NCC * 512], MMDT, kind="ExternalInput")
    wv_d = nc.dram_tensor("wv", [128, NCC * HPC * D], MMDT, kind="ExternalInput")
    wp_d = nc.dram_tensor("wp", [128, 2 * C], MMDT, kind="ExternalInput")
    trig_d = nc.dram_tensor("trig", [128, 2 * T], MMDT, kind="ExternalInput")
    mask_d = nc.dram_tensor("masks", [128, 4 * TCH], MMDT, kind="ExternalInput")
    hsel_d = nc.dram_tensor("hsel", [2, 128], MMDT, kind="ExternalInput")
    out_d = nc.dram_tensor("out", [T, C], MMDT, kind="ExternalOutput")
    if _DEBUG:
        dbg_q0 = nc.dram_tensor("dbg_q0", [128, T], F32, kind="ExternalOutput")
        dbg_k0 = nc.dram_tensor("dbg_k0", [128, T], F32, kind="ExternalOutput")
        dbg_y0 = nc.dram_tensor("dbg_y0", [128, T], F32, kind="ExternalOutput")
        dbg_va = nc.dram_tensor("dbg_va", [128, HPC * (D + 1)], F32, kind="ExternalOutput")
        dbg_et = nc.dram_tensor("dbg_et", [128, 2 * TCH], F32, kind="ExternalOutput")
        dbg_dr = nc.dram_tensor("dbg_dr", [1, 2 * TCH], F32, kind="ExternalOutput")
    if has_battn:
        bqk_d = nc.dram_tensor("bqk", [128, 4], F32, kind="ExternalInput")
        vbias_d = nc.dram_tensor("vbias", [128, HPC * D], MMDT, kind="ExternalInput")

    with TileContext(nc) as tc:
        with (
            tc.tile_pool(name="wsb", bufs=1) as wsb,      # persistent weights/tables
            tc.tile_pool(name="xsb", bufs=4) as xsb,      # streamed xt chunks
            tc.tile_pool(name="qk", bufs=1) as qksb,      # persistent qT/kT/yT/v
            tc.tile_pool(name="rsb", bufs=4) as rsb,      # rope swap staging
            tc.tile_pool(name="esb", bufs=6) as esb,      # exp tiles
            tc.tile_pool(name="nsb", bufs=2) as nsb,      # norm small tiles
            tc.tile_pool(name="osb", bufs=3) as osb,      # out staging
            tc.tile_pool(name="ps", bufs=2, space="PSUM") as ps,
        ):
            # ---- persistent loads (few big DMAs) ----
            wqk_sb = wsb.tile([128, NCC * 512], MMDT, tag="wqk")
            nc.sync.dma_start(out=wqk_sb[:], in_=wqk_d[:])
            wv_sb = wsb.tile([128, NCC * HPC * D], MMDT, tag="wv")
            nc.sync.dma_start(out=wv_sb[:], in_=wv_d[:])
            trig_sb = wsb.tile([128, 2 * T], MMDT, tag="trig")
            nc.sync.dma_start(out=trig_sb[:], in_=trig_d[:])
            mask_sb = wsb.tile([128, 4 * TCH], MMDT, tag="masks")
            nc.sync.dma_start(out=mask_sb[:], in_=mask_d[:])
            ones_sb = wsb.tile([1, 64], MMDT, tag="ones1")
            nc.sync.dma_start(out=ones_sb[:], in_=hsel_d[0:1, 0:64])
            wp_sb = wsb.tile([128, 2 * C], MMDT, tag="wp")
            nc.sync.dma_start(out=wp_sb[:], in_=wp_d[:])
            if has_battn:
                bqk_sb = wsb.tile([128, 4], F32, tag="bqk")
                nc.sync.dma_start(out=bqk_sb[:], in_=bqk_d[:])
                vbias_sb = wsb.tile([128, HPC * D], MMDT, tag="vbias")
                nc.sync.dma_start(out=vbias_sb[:], in_=vbias_d[:])

            # persistent activations
            qTc = [[qksb.tile([128, TCH], MMDT, tag=f"qT{p}_{c}", name=f"qT{p}_{c}")
                    for c in range(NTC)] for p in range(2)]
            kTc = [[qksb.tile([128, TCH], MMDT, tag=f"kT{p}_{c}", name=f"kT{p}_{c}")
                    for c in range(NTC)] for p in range(2)]
            yTc = [[qksb.tile([128, TCH], MMDT, tag=f"yT{p}_{c}", name=f"yT{p}_{c}")
                    for c in range(NTC)] for p in range(2)]
            vaug = [qksb.tile([128, HPC * (D + 1)], MMDT, tag=f"va{tt}", name=f"va{tt}")
                    for tt in range(NTT)]
            # ones columns of v_aug via on-chip memset (cols h*(D+1)+D)
            for tt in range(NTT):
                nc.vector.memset(vaug[tt][:, D::D + 1], 1.0)

            # prefetch all x chunks up front
            xts = []
            for tci in range(NTC):
                xt = xsb.tile([128, NCC * TCH], MMDT, tag="xt", name=f"xt{tci}")
                nc.sync.dma_start(
                    out=xt[:], in_=xt_d[:, tci * NCC * TCH:(tci + 1) * NCC * TCH])
                xts.append(xt)
            for tci in range(NTC):
                # ---- Phase A: qkv projection + RoPE for t-chunk tci ----
                xt = xts[tci]
                qk_dst = [qTc[0][tci], qTc[1][tci], kTc[0][tci], kTc[1][tci]]
                cw = slice(tci * TCH, (tci + 1) * TCH)
                cosc = trig_sb[:, cw]
                sinc = trig_sb[:, T + tci * TCH: T + (tci + 1) * TCH]
                for ft in (2, 0, 3, 1):  # k_p0, q_p0, k_p1, q_p1
                    pqk = ps.tile([128, TCH], F32, tag="pa", name=f"pqk_{tci}_{ft}")
                    for cc in range(NCC):
                        nc.tensor.matmul(
                            pqk[:],
                            wqk_sb[:, cc * 512 + ft * 128: cc * 512 + (ft + 1) * 128],
                            xt[:, cc * TCH:(cc + 1) * TCH],
                            start=(cc == 0), stop=(cc == NCC - 1))
                    Xc = qk_dst[ft][:]
                    if has_battn:
                        nc.scalar.activation(Xc, pqk[:], AF.Identity,
                                             bias=bqk_sb[:, ft:ft + 1])
                    else:
                        nc.vector.tensor_copy(Xc, pqk[:])
                    # RoPE immediately (swap halves via SBUF->SBUF DMAs)
                    X = qk_dst[ft]
                    xs = rsb.tile([128, TCH], MMDT, tag="xswap", name=f"xs_{tci}_{ft}")
                    nc.gpsimd.dma_start(out=xs[0:32, :], in_=X[32:64, :])
                    nc.gpsimd.dma_start(out=xs[32:64, :], in_=X[0:32, :])
                    nc.gpsimd.dma_start(out=xs[64:96, :], in_=X[96:128, :])
                    nc.sync.dma_start(out=xs[96:128, :], in_=X[64:96, :])
                    nc.vector.tensor_tensor(xs[:], xs[:], sinc, ALU.mult)
                    nc.vector.tensor_tensor(Xc, Xc, cosc, ALU.mult)
                    nc.vector.tensor_tensor(Xc, Xc, xs[:], ALU.add)
                # v: two j-pairs, each one PSUM bank
                for jj in range(2):
                    pv = ps.tile([128, 2 * HPC * D], F32, tag="pa", name=f"pv_{tci}_{jj}")
                    for j2 in range(2):
                        j = jj * 2 + j2
                        for cc in range(NCC):
                            nc.tensor.matmul(
                                pv[:, j2 * 256:(j2 + 1) * 256],
                                xt[:, cc * TCH + j * 128: cc * TCH + (j + 1) * 128],
                                wv_sb[:, cc * 256:(cc + 1) * 256],
                                start=(cc == 0), stop=(cc == NCC - 1))
                    for j2 in range(2):
                        tt = tci * 4 + jj * 2 + j2
                        dst = vaug[tt][:, 0:HPC * (D + 1)].rearrange(
                            "p (h e) -> p h e", e=D + 1)[:, :, 0:D]
                        src = pv[:, j2 * 256:(j2 + 1) * 256].rearrange(
                            "p (h e) -> p h e", e=D)
                        if has_battn:
                            nc.vector.scalar_tensor_tensor(
                                dst, src, 0.0,
                                vbias_sb[:].rearrange("p (h e) -> p h e", e=D),
                                ALU.add, ALU.add)
                        else:
                            nc.vector.tensor_copy(dst, src)

                # ---- Phase B: causal attention for q-chunk qc = tci ----
                qc = tci
                nk = 4 * qc + 4
                for p in range(2):
                    yps = [ps.tile([D + 1, TCH], F32, tag="yacc",
                                   name=f"yps_{p}_{qc}_{h}") for h in range(2)]
                    for ktp in range(nk // 2):
                        sc = [ps.tile([128, 2 * TCH], F32, tag="sc",
                                      name=f"sc_{p}_{qc}_{ktp}_{h}") for h in range(2)]
                        et = [esb.tile([128, 2 * TCH], MMDT, tag="et",
                                       name=f"et_{p}_{qc}_{ktp}_{h}") for h in range(2)]
                        for half in range(2):
                            kt = 2 * ktp + half
                            for h in range(2):
                                nc.tensor.matmul(
                                    sc[h][:, half * TCH:(half + 1) * TCH],
                                    kTc[p][kt // 4][h * 64:(h + 1) * 64,
                                                    (kt % 4) * 128:
                                                    (kt % 4 + 1) * 128],
                                    qTc[p][qc][h * 64:(h + 1) * 64, :],
                                    start=True, stop=True,
                                    tile_position=(64 * h, 0))
                        for h in range(2):
                            nc.scalar.activation(et[h][:], sc[h][:], AF.Exp,
                                                 scale=0.125)
                        if _DEBUG and p == 0 and qc == 0 and ktp == 0:
                            dt_ = osb.tile([128, 2 * TCH], F32, tag="dbge")
                            nc.vector.tensor_copy(dt_[:], et[0][:])
                            nc.sync.dma_start(out=dbg_et[:], in_=dt_[:])
                        # causal masking for diagonal-crossing tiles
                        for half in range(2):
                            kt = 2 * ktp + half
                            m = kt - 4 * qc
                            if m >= 0:
                                w = 128 * (m + 1)
                                off = half * TCH
                                for h in range(2):
                                    nc.vector.tensor_tensor(
                                        et[h][:, off:off + w],
                                        et[h][:, off:off + w],
                                        mask_sb[:, m * TCH: m * TCH + w],
                                        ALU.mult)
                        # attn @ v
                        for half in range(2):
                            kt = 2 * ktp + half
                            for h in range(2):
                                hh = 2 * p + h
                                nc.tensor.matmul(
                                    yps[h][:],
                                    vaug[kt][:, hh * (D + 1):(hh + 1) * (D + 1)],
                                    et[h][:, half * TCH:(half + 1) * TCH],
                                    start=(kt == 0), stop=(kt == nk - 1))
                    # normalize: recip = exp(-ln(den)); broadcast via 2 matmuls
                    dln = nsb.tile([1, 2 * TCH], F32, tag="dln", name=f"dln_{p}_{qc}")
                    for h in range(2):
                        nc.scalar.activation(dln[0:1, h * TCH:(h + 1) * TCH],
                                             yps[h][D:D + 1, :], AF.Ln)
                    drec = nsb.tile([1, 2 * TCH], MMDT, tag="drec", name=f"drec_{p}_{qc}")
                    nc.scalar.activation(drec[:], dln[:], AF.Exp, scale=-1.0)
                    pb = ps.tile([128, TCH], F32, tag="pa", name=f"pb_{p}_{qc}")
                    for h in range(2):
                        nc.tensor.matmul(pb[h * 64:(h + 1) * 64, :], ones_sb[:],
                                         drec[0:1, h * TCH:(h + 1) * TCH],
                                         start=True, stop=True,
                                         tile_position=(0, 64 * h))
                    rb = nsb.tile([128, TCH], MMDT, tag="rb", name=f"rb_{p}_{qc}")
                    nc.vector.tensor_copy(rb[:], pb[:])
                    if _DEBUG and p == 0 and qc == 0:
                        dr_ = osb.tile([1, 2 * TCH], F32, tag="dbgr")
                        nc.vector.tensor_copy(dr_[:], dln[:])
                        nc.sync.dma_start(out=dbg_dr[:], in_=dr_[:])
                    for h in range(2):
                        nc.vector.tensor_tensor(
                            yTc[p][qc][h * 64:(h + 1) * 64, :],
                            yps[h][0:D, :], rb[h * 64:(h + 1) * 64, :], ALU.mult)

                # ---- Phase C: output projection for q-chunk qc ----
                for tt in range(4 * qc, 4 * qc + 4):
                    # kk-outer so the stationary yT slice is loaded once
                    # for both output chunks
                    pps = [ps.tile([128, TCH], F32, tag="pa",
                                   name=f"pp_{tt}_{nchunk}")
                           for nchunk in range(2)]
                    for kk in range(2):
                        for nchunk in range(2):
                            nc.tensor.matmul(
                                pps[nchunk][:],
                                yTc[kk][tt // 4][:, (tt % 4) * 128:
                                                 (tt % 4 + 1) * 128],
                                wp_sb[:, kk * C + nchunk * TCH:
                                      kk * C + (nchunk + 1) * TCH],
                                start=(kk == 0), stop=(kk == 1))
                    for nchunk in range(2):
                        ot = osb.tile([128, TCH], MMDT, tag="ot",
                                      name=f"ot_{tt}_{nchunk}")
                        if qc == NTC - 1:
                            nc.scalar.copy(ot[:], pps[nchunk][:])
                        else:
                            nc.vector.tensor_copy(ot[:], pps[nchunk][:])
                        nc.sync.dma_start(
                            out=out_d[tt * 128:(tt + 1) * 128,
                                      nchunk * TCH:(nchunk + 1) * TCH],
                            in_=ot[:])

            if _DEBUG:
                for srcs, dst_t in [(qTc[0], dbg_q0), (kTc[0], dbg_k0),
                                    (yTc[0], dbg_y0)]:
                    for c in range(NTC):
                        dt_ = osb.tile([128, TCH], F32, tag="dbg")
                        nc.vector.tensor_copy(dt_[:], srcs[c][:])
                        nc.sync.dma_start(
                            out=dst_t[:, c * TCH:(c + 1) * TCH], in_=dt_[:])
                dv_ = osb.tile([128, HPC * (D + 1)], F32, tag="dbgv")
                nc.vector.tensor_copy(dv_[:], vaug[0][:])
                nc.sync.dma_start(out=dbg_va[:], in_=dv_[:])

    nc.finalize()
    return nc


def _rope_tables():
    dd = (np.arange(128) % 64) % 32
    fraction = (2.0 * np.arange(32, dtype=np.float32) / 64).astype(np.float32)
    timescale = (np.float32(10000.0) ** fraction).astype(np.float32)
    pos = np.arange(T, dtype=np.float32)
    ang = (pos[None, :] / timescale[dd][:, None]).astype(np.float32)  # [128, T]
    cos_t = np.cos(ang).astype(np.float32)
    sin_t = np.sin(ang).astype(np.float32)
    sgn = np.where((np.arange(128) % 64) < 32, np.float32(-1.0), np.float32(1.0))
    sin_signed = (sin_t * sgn[:, None]).astype(np.float32)
    return cos_t, sin_signed


def _mask_tiles():
    masks = np.zeros((128, 4 * TCH), np.float32)
    r = np.arange(128)[:, None]
    c = np.arange(TCH)[None, :]
    for m in range(4):
        masks[:, m * TCH:(m + 1) * TCH] = (c >= 128 * m + r).astype(np.float32)
    return masks


def kernel(x, W_attn, b_attn, W_proj, b_proj):
    x = np.asarray(x, np.float32)
    W_attn = np.asarray(W_attn, np.float32)
    b_attn = np.asarray(b_attn, np.float32)
    W_proj = np.asarray(W_proj, np.float32)
    b_proj = np.asarray(b_proj, np.float32)

    _patch_act_tables()
    has_battn = bool(np.any(b_attn != 0))
    key = ("v5", has_battn, _DEBUG)
    if key not in _prog_cache:
        _prog_cache[key] = _build_program(has_battn)
    nc = _prog_cache[key]

    import ml_dtypes
    bf = ml_dtypes.bfloat16
    cos_t, sin_signed = _rope_tables()
    trig = np.concatenate([cos_t, sin_signed], axis=1).astype(bf)  # [128, 4096]
    masks = _mask_tiles().astype(bf)
    hsel = np.ones((2, 128), bf)

    in_maps = []
    for core in range(NCORES):
        b, g = divmod(core, HPC)
        hs = [HPC * g + i for i in range(HPC)]
        qkcols = []
        for base in (0, C):  # q tiles then k tiles
            for p in range(2):
                for i in (2 * p, 2 * p + 1):
                    qkcols += [base + hs[i] * D + d for d in range(D)]
        vcols = [2 * C + h * D + d for h in hs for d in range(D)]
        rows = [h * D + d for h in hs for d in range(D)]

        wqk = np.ascontiguousarray(
            W_attn[:, qkcols].reshape(NCC, 128, 512).transpose(1, 0, 2)
            .reshape(128, NCC * 512)).astype(bf)
        wv = np.ascontiguousarray(
            W_attn[:, vcols].reshape(NCC, 128, 256).transpose(1, 0, 2)
            .reshape(128, NCC * 256)).astype(bf)
        wp = np.ascontiguousarray(
            W_proj[rows, :].reshape(2, 128, C).transpose(1, 0, 2)
            .reshape(128, 2 * C)).astype(bf)
        xt = np.ascontiguousarray(
            x[b].reshape(NTC, TCH, NCC, 128).transpose(3, 0, 2, 1)
            .reshape(128, NTC * NCC * TCH)).astype(bf)

        im = {
            "xt": xt, "wqk": wqk, "wv": wv, "wp": wp,
            "trig": trig, "masks": masks, "hsel": hsel,
        }
        if has_battn:
            im["bqk"] = np.ascontiguousarray(
                b_attn[qkcols].reshape(4, 128).T).astype(np.float32)
            im["vbias"] = np.tile(b_attn[vcols], (128, 1)).astype(bf)
        in_maps.append(im)

    trace = bool(os.environ.get("TRNK_TRACE"))
    if trace:
        try:
            import ntff_shim  # noqa: F401
        except ImportError:
            trace = False
    res = run_bass_kernel_spmd(nc, in_maps, list(range(NCORES)), trace=trace,
                               tmpdir=os.environ.get("TRNK_TMPDIR") or None)
    if trace:
        globals()["_last_exec_time_ns"] = res.exec_time_ns
        globals()["_last_result"] = res

    globals()["_dbg_results"] = res.results
    out = np.zeros((B, T, C), np.float32)
    for core in range(NCORES):
        b = core // HPC
        out[b] += np.asarray(res.results[core]["out"], np.float32)
    out += b_proj[None, None, :]
    return out


# revision 25
# speedup vs baseline: 1.0417x; 1.0366x over previous
"""Causal self-attention (B=2, T=2048, C=1024, H=16, D=64) on 8 TRN2 cores.

Sharding: core = b*4 + g handles batch b, heads 4g..4g+3 (data parallel on B,
tensor parallel on heads). Each core computes its 4 heads' contribution to
x @ W_proj; host sums the 4 partial outputs per batch and adds b_proj.

v2 redesign vs the 341us baseline (which was PE-cold/ACT-thrash bound):
  - All inputs host-pre-laid-out to SBUF layout -> 9 big contiguous DMAs
    (baseline: 74 small DMAs x ~650ns serial issue = 50us dead prologue).
  - Single software pipeline over tci: A(tci) qkv-proj + per-chunk RoPE,
    then B(p, qc=tci) flash attention, then C(qc=tci) out-proj, so the
    Tile scheduler can overlap A(tci+1) PE work under B(tci) ACT exp and
    keep the PE HAM-warm throughout.
  - exp/ln pinned to the natural_log_exp_and_others ACT table set via a
    get_activation_tables monkeypatch (baseline thrashed 19 table loads).
  - RoPE swap-halves fused into the sin-multiply with partition-block
    strided APs (2 TTs), PSUM evacuations on DVE, softmax denominators
    ln/exp'd on ACT straight out of PSUM, one broadcast matmul per (p,qc).
  - bf16 everywhere off-chip except nothing: out is bf16, host accumulates
    in fp32.
"""
import os
import numpy as np

import concourse.bass as bass
import concourse.mybir as mybir
from concourse import bacc
from concourse.tile import TileContext
from concourse.bass_utils import run_bass_kernel_spmd

B, T, C, H, D = 2, 2048, 1024, 16, 64
HPC = 4          # heads per core
NCORES = 8
TCH = 512        # t-chunk / q-chunk width
NTC = T // TCH   # 4
NTT = T // 128   # 16 t-tiles
NCC = C // 128   # 8 c-chunks
F32 = mybir.dt.float32
BF16 = mybir.dt.bfloat16
MMDT = BF16
AF = mybir.ActivationFunctionType
ALU = mybir.AluOpType

_prog_cache = {}
_DEBUG = bool(os.environ.get("TRNK_DEBUG"))


def _patch_act_tables():
    """Make every exp/ln activation resolve to natural_log_exp_and_others
    so the kernel needs exactly one ACT table load (the baseline thrashed
    19 loads between exp_and_others and natural_log)."""
    try:
        import concourse.hw_specs as _hw
        import concourse.bacc as _bc
        orig = _hw.get_activation_tables
        if getattr(orig, "_trnk_patched", False):
            return

        def patched(arch):
            tabs = orig(arch)
            keep = "natural_log_exp_and_others"
            if keep in tabs:
                for name, s in tabs.items():
                    if name != keep:
                        s.discard(AF.Exp)
                        s.discard(AF.Ln)
            return tabs

        patched._trnk_patched = True
        _hw.get_activation_tables = patched
        _bc.get_activation_tables = patched
        try:
            import concourse.bass_interp as _bi
            _bi.get_activation_tables = patched
        except Exception:
            pass
    except Exception:
        pass


def _blk(ap):
    """View a [128, N] AP as partition blocks of 32: [2, 2, 32, N]
    (index [a, b]: partitions a*64 + b*32 + 0..31)."""
    return ap.rearrange("(a b s) t -> a b s t", a=2, b=2, s=32)


def _emit_c(nc, ps, osb, yTc, wp_sb, out_d, qc, last):
    """Output projection for q-chunk qc (kk-outer: stationary yT slice
    loaded once for both output column chunks)."""
    for tt in range(4 * qc, 4 * qc + 4):
        pps = [ps.tile([128, TCH], F32, tag="pa", name=f"pp_{tt}_{nchunk}")
               for nchunk in range(2)]
        for kk in range(2):
            for nchunk in range(2):
                nc.tensor.matmul(
                    pps[nchunk][:],
                    yTc[kk][tt // 4][:, (tt % 4) * 128:(tt % 4 + 1) * 128],
                    wp_sb[:, kk * C + nchunk * TCH: kk * C + (nchunk + 1) * TCH],
                    start=(kk == 0), stop=(kk == 1))
        for nchunk in range(2):
            ot = osb.tile([128, TCH], MMDT, tag="ot", name=f"ot_{tt}_{nchunk}")
            if last:
                nc.scalar.copy(ot[:], pps[nchunk][:])
            else:
                nc.vector.tensor_copy(ot[:], pps[nchunk][:])
            nc.sync.dma_start(
                out=out_d[tt * 128:(tt + 1) * 128,
                          nchunk * TCH:(nchunk + 1) * TCH],
                in_=ot[:])


def _build_program(has_battn: bool):
    nc = bacc.Bacc("TRN2", target_bir_lowering=False, debug=False,
                   num_devices=NCORES)
    # ---- DRAM I/O (per core, all pre-laid-out to SBUF layout) ----
    xt_d = nc.dram_tensor("xt", [128, NTC * NCC * TCH], MMDT, kind="ExternalInput")
    wqk_d = nc.dram_tensor("wqk", [128, NCC * 512], MMDT, kind="ExternalInput")
    wv_d = nc.dram_tensor("wv", [128, NCC * HPC * D], MMDT, kind="ExternalInput")
    wp_d = nc.dram_tensor("wp", [128, 2 * C], MMDT, kind="ExternalInput")
    trig_d = nc.dram_tensor("trig", [128, 2 * T], MMDT, kind="ExternalInput")
    mask_d = nc.dram_tensor("masks", [128, 4 * TCH], MMDT, kind="ExternalInput")
    hsel_d = nc.dram_tensor("hsel", [2, 128], MMDT, kind="ExternalInput")
    out_d = nc.dram_tensor("out", [T, C], MMDT, kind="ExternalOutput")
    if _DEBUG:
        dbg_q0 = nc.dram_tensor("dbg_q0", [128, T], F32, kind="ExternalOutput")
        dbg_k0 = nc.dram_tensor("dbg_k0", [128, T], F32, kind="ExternalOutput")
        dbg_y0 = nc.dram_tensor("dbg_y0", [128, T], F32, kind="ExternalOutput")
        dbg_va = nc.dram_tensor("dbg_va", [128, HPC * (D + 1)], F32, kind="ExternalOutput")
        dbg_et = nc.dram_tensor("dbg_et", [128, 2 * TCH], F32, kind="ExternalOutput")
        dbg_dr = nc.dram_tensor("dbg_dr", [1, 2 * TCH], F32, kind="ExternalOutput")
    if has_battn:
        bqk_d = nc.dram_tensor("bqk", [128, 4], F32, kind="ExternalInput")
        vbias_d = nc.dram_tensor("vbias", [128, HPC * D], MMDT, kind="ExternalInput")

    with TileContext(nc) as tc:
        with (
            tc.tile_pool(name="wsb", bufs=1) as wsb,      # persistent weights/tables
            tc.tile_pool(name="xsb", bufs=4) as xsb,      # streamed xt chunks
            tc.tile_pool(name="qk", bufs=1) as qksb,      # persistent qT/kT/yT/v
            tc.tile_pool(name="rsb", bufs=4) as rsb,      # rope swap staging
            tc.tile_pool(name="esb", bufs=6) as esb,      # exp tiles
            tc.tile_pool(name="nsb", bufs=2) as nsb,      # norm small tiles
            tc.tile_pool(name="osb", bufs=3) as osb,      # out staging
            tc.tile_pool(name="ps", bufs=2, space="PSUM") as ps,
        ):
            # ---- persistent loads (few big DMAs) ----
            wqk_sb = wsb.tile([128, NCC * 512], MMDT, tag="wqk")
            nc.sync.dma_start(out=wqk_sb[:], in_=wqk_d[:])
            wv_sb = wsb.tile([128, NCC * HPC * D], MMDT, tag="wv")
            nc.sync.dma_start(out=wv_sb[:], in_=wv_d[:])
            trig_sb = wsb.tile([128, 2 * T], MMDT, tag="trig")
            nc.sync.dma_start(out=trig_sb[:], in_=trig_d[:])
            mask_sb = wsb.tile([128, 4 * TCH], MMDT, tag="masks")
            nc.sync.dma_start(out=mask_sb[:], in_=mask_d[:])
            ones_sb = wsb.tile([1, 64], MMDT, tag="ones1")
            nc.sync.dma_start(out=ones_sb[:], in_=hsel_d[0:1, 0:64])
            wp_sb = wsb.tile([128, 2 * C], MMDT, tag="wp")
            nc.sync.dma_start(out=wp_sb[:], in_=wp_d[:])
            if has_battn:
                bqk_sb = wsb.tile([128, 4], F32, tag="bqk")
                nc.sync.dma_start(out=bqk_sb[:], in_=bqk_d[:])
                vbias_sb = wsb.tile([128, HPC * D], MMDT, tag="vbias")
                nc.sync.dma_start(out=vbias_sb[:], in_=vbias_d[:])

            # persistent activations
            qTc = [[qksb.tile([128, TCH], MMDT, tag=f"qT{p}_{c}", name=f"qT{p}_{c}")
                    for c in range(NTC)] for p in range(2)]
            kTc = [[qksb.tile([128, TCH], MMDT, tag=f"kT{p}_{c}", name=f"kT{p}_{c}")
                    for c in range(NTC)] for p in range(2)]
            yTc = [[qksb.tile([128, TCH], MMDT, tag=f"yT{p}_{c}", name=f"yT{p}_{c}")
                    for c in range(NTC)] for p in range(2)]
            vaug = [qksb.tile([128, HPC * (D + 1)], MMDT, tag=f"va{tt}", name=f"va{tt}")
                    for tt in range(NTT)]
            # ones columns of v_aug via on-chip memset (cols h*(D+1)+D)
            for tt in range(NTT):
                nc.vector.memset(vaug[tt][:, D::D + 1], 1.0)

            # HAM warmup: dummy matmul chain while input DMAs land
            wdum = wsb.tile([128, 128], MMDT, tag="wdum")
            nc.vector.memset(wdum[:], 0.0)
            pwarm = ps.tile([128, TCH], F32, tag="pa", name="pwarm")
            for i in range(60):
                nc.tensor.matmul(pwarm[:, 0:128], wdum[:], wdum[:],
                                 start=(i == 0), stop=(i == 59))

            # prefetch all x chunks up front
            xts = []
            for tci in range(NTC):
                xt = xsb.tile([128, NCC * TCH], MMDT, tag="xt", name=f"xt{tci}")
                nc.sync.dma_start(
                    out=xt[:], in_=xt_d[:, tci * NCC * TCH:(tci + 1) * NCC * TCH])
                xts.append(xt)
            for tci in range(NTC):
                # ---- Phase A: qkv projection + RoPE for t-chunk tci ----
                xt = xts[tci]
                qk_dst = [qTc[0][tci], qTc[1][tci], kTc[0][tci], kTc[1][tci]]
                cw = slice(tci * TCH, (tci + 1) * TCH)
                cosc = trig_sb[:, cw]
                sinc = trig_sb[:, T + tci * TCH: T + (tci + 1) * TCH]
                for ft in (2, 0, 3, 1):  # k_p0, q_p0, k_p1, q_p1
                    pqk = ps.tile([128, TCH], F32, tag="pa", name=f"pqk_{tci}_{ft}")
                    for cc in range(NCC):
                        nc.tensor.matmul(
                            pqk[:],
                            wqk_sb[:, cc * 512 + ft * 128: cc * 512 + (ft + 1) * 128],
                            xt[:, cc * TCH:(cc + 1) * TCH],
                            start=(cc == 0), stop=(cc == NCC - 1))
                    Xc = qk_dst[ft][:]
                    if has_battn:
                        nc.scalar.activation(Xc, pqk[:], AF.Identity,
                                             bias=bqk_sb[:, ft:ft + 1])
                    else:
                        nc.vector.tensor_copy(Xc, pqk[:])
                    # RoPE immediately (swap halves via SBUF->SBUF DMAs)
                    X = qk_dst[ft]
                    xs = rsb.tile([128, TCH], MMDT, tag="xswap", name=f"xs_{tci}_{ft}")
                    nc.gpsimd.dma_start(out=xs[0:32, :], in_=X[32:64, :])
                    nc.gpsimd.dma_start(out=xs[32:64, :], in_=X[0:32, :])
                    nc.gpsimd.dma_start(out=xs[64:96, :], in_=X[96:128, :])
                    nc.sync.dma_start(out=xs[96:128, :], in_=X[64:96, :])
                    nc.vector.tensor_tensor(xs[:], xs[:], sinc, ALU.mult)
                    nc.vector.tensor_tensor(Xc, Xc, cosc, ALU.mult)
                    nc.vector.tensor_tensor(Xc, Xc, xs[:], ALU.add)
                # v: two j-pairs, each one PSUM bank
                for jj in range(2):
                    pv = ps.tile([128, 2 * HPC * D], F32, tag="pa", name=f"pv_{tci}_{jj}")
                    for j2 in range(2):
                        j = jj * 2 + j2
                        for cc in range(NCC):
                            nc.tensor.matmul(
                                pv[:, j2 * 256:(j2 + 1) * 256],
                                xt[:, cc * TCH + j * 128: cc * TCH + (j + 1) * 128],
                                wv_sb[:, cc * 256:(cc + 1) * 256],
                                start=(cc == 0), stop=(cc == NCC - 1))
                    for j2 in range(2):
                        tt = tci * 4 + jj * 2 + j2
                        dst = vaug[tt][:, 0:HPC * (D + 1)].rearrange(
                            "p (h e) -> p h e", e=D + 1)[:, :, 0:D]
                        src = pv[:, j2 * 256:(j2 + 1) * 256].rearrange(
                            "p (h e) -> p h e", e=D)
                        if has_battn:
                            nc.vector.scalar_tensor_tensor(
                                dst, src, 0.0,
                                vbias_sb[:].rearrange("p (h e) -> p h e", e=D),
                                ALU.add, ALU.add)
                        else:
                            nc.vector.tensor_copy(dst, src)

                # ---- Phase C for the previous q-chunk (emitted here so its
                # matmuls fill the PE while this chunk's RoPE completes) ----
                if tci > 0:
                    _emit_c(nc, ps, osb, yTc, wp_sb, out_d, tci - 1, False)

                # ---- Phase B: causal attention for q-chunk qc = tci ----
                qc = tci
                nk = 4 * qc + 4
                for p in range(2):
                    yps = [ps.tile([D + 1, TCH], F32, tag="yacc",
                                   name=f"yps_{p}_{qc}_{h}") for h in range(2)]
                    for ktp in range(nk // 2):
                        sc = [ps.tile([128, 2 * TCH], F32, tag="sc",
                                      name=f"sc_{p}_{qc}_{ktp}_{h}") for h in range(2)]
                        et = [esb.tile([128, 2 * TCH], MMDT, tag="et",
                                       name=f"et_{p}_{qc}_{ktp}_{h}") for h in range(2)]
                        for half in range(2):
                            kt = 2 * ktp + half
                            for h in range(2):
                                nc.tensor.matmul(
                                    sc[h][:, half * TCH:(half + 1) * TCH],
                                    kTc[p][kt // 4][h * 64:(h + 1) * 64,
                                                    (kt % 4) * 128:
                                                    (kt % 4 + 1) * 128],
                                    qTc[p][qc][h * 64:(h + 1) * 64, :],
                                    start=True, stop=True,
                                    tile_position=(64 * h, 0))
                        for h in range(2):
                            nc.scalar.activation(et[h][:], sc[h][:], AF.Exp,
                                                 scale=0.125)
                        if _DEBUG and p == 0 and qc == 0 and ktp == 0:
                            dt_ = osb.tile([128, 2 * TCH], F32, tag="dbge")
                            nc.vector.tensor_copy(dt_[:], et[0][:])
                            nc.sync.dma_start(out=dbg_et[:], in_=dt_[:])
                        # causal masking for diagonal-crossing tiles
                        for half in range(2):
                            kt = 2 * ktp + half
                            m = kt - 4 * qc
                            if m >= 0:
                                w = 128 * (m + 1)
                                off = half * TCH
                                for h in range(2):
                                    nc.vector.tensor_tensor(
                                        et[h][:, off:off + w],
                                        et[h][:, off:off + w],
                                        mask_sb[:, m * TCH: m * TCH + w],
                                        ALU.mult)
                        # attn @ v
                        for half in range(2):
                            kt = 2 * ktp + half
                            for h in range(2):
                                hh = 2 * p + h
                                nc.tensor.matmul(
                                    yps[h][:],
                                    vaug[kt][:, hh * (D + 1):(hh + 1) * (D + 1)],
                                    et[h][:, half * TCH:(half + 1) * TCH],
                                    start=(kt == 0), stop=(kt == nk - 1))
                    # normalize: recip = exp(-ln(den)); broadcast via 2 matmuls
                    dln = nsb.tile([1, 2 * TCH], F32, tag="dln", name=f"dln_{p}_{qc}")
                    for h in range(2):
                        nc.scalar.activation(dln[0:1, h * TCH:(h + 1) * TCH],
                                             yps[h][D:D + 1, :], AF.Ln)
                    drec = nsb.tile([1, 2 * TCH], MMDT, tag="drec", name=f"drec_{p}_{qc}")
                    nc.scalar.activation(drec[:], dln[:], AF.Exp, scale=-1.0)
                    pb = ps.tile([128, TCH], F32, tag="pa", name=f"pb_{p}_{qc}")
                    for h in range(2):
                        nc.tensor.matmul(pb[h * 64:(h + 1) * 64, :], ones_sb[:],
                                         drec[0:1, h * TCH:(h + 1) * TCH],
                                         start=True, stop=True,
                                         tile_position=(0, 64 * h))
                    rb = nsb.tile([128, TCH], MMDT, tag="rb", name=f"rb_{p}_{qc}")
                    nc.vector.tensor_copy(rb[:], pb[:])
                    if _DEBUG and p == 0 and qc == 0:
                        dr_ = osb.tile([1, 2 * TCH], F32, tag="dbgr")
                        nc.vector.tensor_copy(dr_[:], dln[:])
                        nc.sync.dma_start(out=dbg_dr[:], in_=dr_[:])
                    for h in range(2):
                        nc.vector.tensor_tensor(
                            yTc[p][qc][h * 64:(h + 1) * 64, :],
                            yps[h][0:D, :], rb[h * 64:(h + 1) * 64, :], ALU.mult)

            # final q-chunk's output projection
            _emit_c(nc, ps, osb, yTc, wp_sb, out_d, NTC - 1, True)

            if _DEBUG:
                for srcs, dst_t in [(qTc[0], dbg_q0), (kTc[0], dbg_k0),
                                    (yTc[0], dbg_y0)]:
                    for c in range(NTC):
                        dt_ = osb.tile([128, TCH], F32, tag="dbg")
                        nc.vector.tensor_copy(dt_[:], srcs[c][:])
                        nc.sync.dma_start(
                            out=dst_t[:, c * TCH:(c + 1) * TCH], in_=dt_[:])
                dv_ = osb.tile([128, HPC * (D + 1)], F32, tag="dbgv")
                nc.vector.tensor_copy(dv_[:], vaug[0][:])
                nc.sync.dma_start(out=dbg_va[:], in_=dv_[:])

    nc.finalize()
    return nc


def _rope_tables():
    dd = (np.arange(128) % 64) % 32
    fraction = (2.0 * np.arange(32, dtype=np.float32) / 64).astype(np.float32)
    timescale = (np.float32(10000.0) ** fraction).astype(np.float32)
    pos = np.arange(T, dtype=np.float32)
    ang = (pos[None, :] / timescale[dd][:, None]).astype(np.float32)  # [128, T]
    cos_t = np.cos(ang).astype(np.float32)
    sin_t = np.sin(ang).astype(np.float32)
    sgn = np.where((np.arange(128) % 64) < 32, np.float32(-1.0), np.float32(1.0))
    sin_signed = (sin_t * sgn[:, None]).astype(np.float32)
    return cos_t, sin_signed


def _mask_tiles():
    masks = np.zeros((128, 4 * TCH), np.float32)
    r = np.arange(128)[:, None]
    c = np.arange(TCH)[None, :]
    for m in range(4):
        masks[:, m * TCH:(m + 1) * TCH] = (c >= 128 * m + r).astype(np.float32)
    return masks


def kernel(x, W_attn, b_attn, W_proj, b_proj):
    x = np.asarray(x, np.float32)
    W_attn = np.asarray(W_attn, np.float32)
    b_attn = np.asarray(b_attn, np.float32)
    W_proj = np.asarray(W_proj, np.float32)
    b_proj = np.asarray(b_proj, np.float32)

    _patch_act_tables()
    has_battn = bool(np.any(b_attn != 0))
    key = ("v5", has_battn, _DEBUG)
    if key not in _prog_cache:
        _prog_cache[key] = _build_program(has_battn)
    nc = _prog_cache[key]

    import ml_dtypes
    bf = ml_dtypes.bfloat16
    cos_t, sin_signed = _rope_tables()
    trig = np.concatenate([cos_t, sin_signed], axis=1).astype(bf)  # [128, 4096]
    masks = _mask_tiles().astype(bf)
    hsel = np.ones((2, 128), bf)

    in_maps = []
    for core in range(NCORES):
        b, g = divmod(core, HPC)
        hs = [HPC * g + i for i in range(HPC)]
        qkcols = []
        for base in (0, C):  # q tiles then k tiles
            for p in range(2):
                for i in (2 * p, 2 * p + 1):
                    qkcols += [base + hs[i] * D + d for d in range(D)]
        vcols = [2 * C + h * D + d for h in hs for d in range(D)]
        rows = [h * D + d for h in hs for d in range(D)]

        wqk = np.ascontiguousarray(
            W_attn[:, qkcols].reshape(NCC, 128, 512).transpose(1, 0, 2)
            .reshape(128, NCC * 512)).astype(bf)
        wv = np.ascontiguousarray(
            W_attn[:, vcols].reshape(NCC, 128, 256).transpose(1, 0, 2)
            .reshape(128, NCC * 256)).astype(bf)
        wp = np.ascontiguousarray(
            W_proj[rows, :].reshape(2, 128, C).transpose(1, 0, 2)
            .reshape(128, 2 * C)).astype(bf)
        xt = np.ascontiguousarray(
            x[b].reshape(NTC, TCH, NCC, 128).transpose(3, 0, 2, 1)
            .reshape(128, NTC * NCC * TCH)).astype(bf)

        im = {
            "xt": xt, "wqk": wqk, "wv": wv, "wp": wp,
            "trig": trig, "masks": masks, "hsel": hsel,
        }
        if has_battn:
            im["bqk"] = np.ascontiguousarray(
                b_attn[qkcols].reshape(4, 128).T).astype(np.float32)
            im["vbias"] = np.tile(b_attn[vcols], (128, 1)).astype(bf)
        in_maps.append(im)

    trace = bool(os.environ.get("TRNK_TRACE"))
    if trace:
        try:
            import ntff_shim  # noqa: F401
        except ImportError:
            trace = False
    res = run_bass_kernel_spmd(nc, in_maps, list(range(NCORES)), trace=trace,
                               tmpdir=os.environ.get("TRNK_TMPDIR") or None)
    if trace:
        globals()["_last_exec_time_ns"] = res.exec_time_ns
        globals()["_last_result"] = res

    globals()["_dbg_results"] = res.results
    out = np.zeros((B, T, C), np.float32)
    for core in range(NCORES):
        b = core // HPC
        out[b] += np.asarray(res.results[core]["out"], np.float32)
    out += b_proj[None, None, :]
    return out


# revision 26
# speedup vs baseline: 1.1128x; 1.0682x over previous
"""Causal self-attention (B=2, T=2048, C=1024, H=16, D=64) on 8 TRN2 cores.

Sharding: core = b*4 + g handles batch b, heads 4g..4g+3 (data parallel on B,
tensor parallel on heads). Each core computes its 4 heads' contribution to
x @ W_proj; host sums the 4 partial outputs per batch and adds b_proj.

v2 redesign vs the 341us baseline (which was PE-cold/ACT-thrash bound):
  - All inputs host-pre-laid-out to SBUF layout -> 9 big contiguous DMAs
    (baseline: 74 small DMAs x ~650ns serial issue = 50us dead prologue).
  - Single software pipeline over tci: A(tci) qkv-proj + per-chunk RoPE,
    then B(p, qc=tci) flash attention, then C(qc=tci) out-proj, so the
    Tile scheduler can overlap A(tci+1) PE work under B(tci) ACT exp and
    keep the PE HAM-warm throughout.
  - exp/ln pinned to the natural_log_exp_and_others ACT table set via a
    get_activation_tables monkeypatch (baseline thrashed 19 table loads).
  - RoPE swap-halves fused into the sin-multiply with partition-block
    strided APs (2 TTs), PSUM evacuations on DVE, softmax denominators
    ln/exp'd on ACT straight out of PSUM, one broadcast matmul per (p,qc).
  - bf16 everywhere off-chip except nothing: out is bf16, host accumulates
    in fp32.
"""
import os
import numpy as np

import concourse.bass as bass
import concourse.mybir as mybir
from concourse import bacc
from concourse.tile import TileContext
from concourse.bass_utils import run_bass_kernel_spmd

B, T, C, H, D = 2, 2048, 1024, 16, 64
HPC = 4          # heads per core
NCORES = 8
TCH = 512        # t-chunk / q-chunk width
NTC = T // TCH   # 4
NTT = T // 128   # 16 t-tiles
NCC = C // 128   # 8 c-chunks
F32 = mybir.dt.float32
BF16 = mybir.dt.bfloat16
MMDT = BF16
AF = mybir.ActivationFunctionType
ALU = mybir.AluOpType

_prog_cache = {}
_DEBUG = bool(os.environ.get("TRNK_DEBUG"))


def _patch_act_tables():
    """Make every exp/ln activation resolve to natural_log_exp_and_others
    so the kernel needs exactly one ACT table load (the baseline thrashed
    19 loads between exp_and_others and natural_log)."""
    try:
        import concourse.hw_specs as _hw
        import concourse.bacc as _bc
        orig = _hw.get_activation_tables
        if getattr(orig, "_trnk_patched", False):
            return

        def patched(arch):
            tabs = orig(arch)
            keep = "natural_log_exp_and_others"
            if keep in tabs:
                for name, s in tabs.items():
                    if name != keep:
                        s.discard(AF.Exp)
                        s.discard(AF.Ln)
            return tabs

        patched._trnk_patched = True
        _hw.get_activation_tables = patched
        _bc.get_activation_tables = patched
        try:
            import concourse.bass_interp as _bi
            _bi.get_activation_tables = patched
        except Exception:
            pass
    except Exception:
        pass


def _blk(ap):
    """View a [128, N] AP as partition blocks of 32: [2, 2, 32, N]
    (index [a, b]: partitions a*64 + b*32 + 0..31)."""
    return ap.rearrange("(a b s) t -> a b s t", a=2, b=2, s=32)


def _emit_c(nc, ps, osb, yTc, wp_sb, out_d, qc, last):
    """Output projection for q-chunk qc (kk-outer: stationary yT slice
    loaded once for both output column chunks)."""
    for tt in range(4 * qc, 4 * qc + 4):
        pps = [ps.tile([128, TCH], F32, tag="pa", name=f"pp_{tt}_{nchunk}")
               for nchunk in range(2)]
        for kk in range(2):
            for nchunk in range(2):
                nc.tensor.matmul(
                    pps[nchunk][:],
                    yTc[kk][tt // 4][:, (tt % 4) * 128:(tt % 4 + 1) * 128],
                    wp_sb[:, kk * C + nchunk * TCH: kk * C + (nchunk + 1) * TCH],
                    start=(kk == 0), stop=(kk == 1))
        for nchunk in range(2):
            ot = osb.tile([128, TCH], MMDT, tag="ot", name=f"ot_{tt}_{nchunk}")
            if last:
                nc.scalar.copy(ot[:], pps[nchunk][:])
            else:
                nc.vector.tensor_copy(ot[:], pps[nchunk][:])
            nc.sync.dma_start(
                out=out_d[tt * 128:(tt + 1) * 128,
                          nchunk * TCH:(nchunk + 1) * TCH],
                in_=ot[:])


def _build_program(has_battn: bool):
    nc = bacc.Bacc("TRN2", target_bir_lowering=False, debug=False,
                   num_devices=NCORES)
    # ---- DRAM I/O (per core, all pre-laid-out to SBUF layout) ----
    xt_d = nc.dram_tensor("xt", [128, NTC * NCC * TCH], MMDT, kind="ExternalInput")
    wqk_d = nc.dram_tensor("wqk", [128, NCC * 512], MMDT, kind="ExternalInput")
    wv_d = nc.dram_tensor("wv", [128, NCC * HPC * D], MMDT, kind="ExternalInput")
    wp_d = nc.dram_tensor("wp", [128, 2 * C], MMDT, kind="ExternalInput")
    trig_d = nc.dram_tensor("trig", [128, 2 * T], MMDT, kind="ExternalInput")
    mask_d = nc.dram_tensor("masks", [128, 4 * TCH], MMDT, kind="ExternalInput")
    hsel_d = nc.dram_tensor("hsel", [2, 128], MMDT, kind="ExternalInput")
    out_d = nc.dram_tensor("out", [T, C], MMDT, kind="ExternalOutput")
    if _DEBUG:
        dbg_q0 = nc.dram_tensor("dbg_q0", [128, T], F32, kind="ExternalOutput")
        dbg_k0 = nc.dram_tensor("dbg_k0", [128, T], F32, kind="ExternalOutput")
        dbg_y0 = nc.dram_tensor("dbg_y0", [128, T], F32, kind="ExternalOutput")
        dbg_va = nc.dram_tensor("dbg_va", [128, HPC * (D + 1)], F32, kind="ExternalOutput")
        dbg_et = nc.dram_tensor("dbg_et", [128, 2 * TCH], F32, kind="ExternalOutput")
        dbg_dr = nc.dram_tensor("dbg_dr", [1, 2 * TCH], F32, kind="ExternalOutput")
    if has_battn:
        bqk_d = nc.dram_tensor("bqk", [128, 4], F32, kind="ExternalInput")
        vbias_d = nc.dram_tensor("vbias", [128, HPC * D], MMDT, kind="ExternalInput")

    with TileContext(nc) as tc:
        with (
            tc.tile_pool(name="wsb", bufs=1) as wsb,      # persistent weights/tables
            tc.tile_pool(name="xsb", bufs=4) as xsb,      # streamed xt chunks
            tc.tile_pool(name="qk", bufs=1) as qksb,      # persistent qT/kT/yT/v
            tc.tile_pool(name="rsb", bufs=4) as rsb,      # rope swap staging
            tc.tile_pool(name="esb", bufs=6) as esb,      # exp tiles
            tc.tile_pool(name="nsb", bufs=2) as nsb,      # norm small tiles
            tc.tile_pool(name="osb", bufs=3) as osb,      # out staging
            tc.tile_pool(name="ps", bufs=2, space="PSUM") as ps,
        ):
            # ---- persistent loads; issue order = first-use order ----
            wqk_sb = wsb.tile([128, NCC * 512], MMDT, tag="wqk")
            wv_sb = wsb.tile([128, NCC * HPC * D], MMDT, tag="wv")
            trig_sb = wsb.tile([128, 2 * T], MMDT, tag="trig")
            mask_sb = wsb.tile([128, 4 * TCH], MMDT, tag="masks")
            ones_sb = wsb.tile([1, 64], MMDT, tag="ones1")
            wp_sb = wsb.tile([128, 2 * C], MMDT, tag="wp")
            xts = [xsb.tile([128, NCC * TCH], MMDT, tag="xt", name=f"xt{i}")
                   for i in range(NTC)]
            nc.sync.dma_start(out=wqk_sb[:], in_=wqk_d[:])
            nc.sync.dma_start(out=xts[0][:], in_=xt_d[:, 0:NCC * TCH])
            nc.sync.dma_start(out=trig_sb[:], in_=trig_d[:])
            nc.sync.dma_start(out=wv_sb[:], in_=wv_d[:])
            nc.sync.dma_start(out=xts[1][:], in_=xt_d[:, NCC * TCH:2 * NCC * TCH])
            nc.sync.dma_start(out=mask_sb[:], in_=mask_d[:])
            nc.sync.dma_start(out=ones_sb[:], in_=hsel_d[0:1, 0:64])
            nc.sync.dma_start(out=wp_sb[:], in_=wp_d[:])
            for i in (2, 3):
                nc.sync.dma_start(
                    out=xts[i][:], in_=xt_d[:, i * NCC * TCH:(i + 1) * NCC * TCH])
            if has_battn:
                bqk_sb = wsb.tile([128, 4], F32, tag="bqk")
                nc.sync.dma_start(out=bqk_sb[:], in_=bqk_d[:])
                vbias_sb = wsb.tile([128, HPC * D], MMDT, tag="vbias")
                nc.sync.dma_start(out=vbias_sb[:], in_=vbias_d[:])

            # persistent activations
            qTc = [[qksb.tile([128, TCH], MMDT, tag=f"qT{p}_{c}", name=f"qT{p}_{c}")
                    for c in range(NTC)] for p in range(2)]
            kTc = [[qksb.tile([128, TCH], MMDT, tag=f"kT{p}_{c}", name=f"kT{p}_{c}")
                    for c in range(NTC)] for p in range(2)]
            yTc = [[qksb.tile([128, TCH], MMDT, tag=f"yT{p}_{c}", name=f"yT{p}_{c}")
                    for c in range(NTC)] for p in range(2)]
            vaug = [qksb.tile([128, HPC * (D + 1)], MMDT, tag=f"va{tt}", name=f"va{tt}")
                    for tt in range(NTT)]
            # ones columns of v_aug via on-chip memset (cols h*(D+1)+D)
            for tt in range(NTT):
                nc.vector.memset(vaug[tt][:, D::D + 1], 1.0)

            # HAM warmup: dummy matmul chain while input DMAs land
            wdum = wsb.tile([128, 128], MMDT, tag="wdum")
            nc.vector.memset(wdum[:], 0.0)
            pwarm = ps.tile([128, TCH], F32, tag="pa", name="pwarm")
            for i in range(60):
                nc.tensor.matmul(pwarm[:, 0:128], wdum[:], wdum[:],
                                 start=(i == 0), stop=(i == 59))

            def emit_a(tci):
                # ---- Phase A: qkv projection + RoPE for t-chunk tci ----
                xt = xts[tci]
                qk_dst = [qTc[0][tci], qTc[1][tci], kTc[0][tci], kTc[1][tci]]
                cw = slice(tci * TCH, (tci + 1) * TCH)
                cosc = trig_sb[:, cw]
                sinc = trig_sb[:, T + tci * TCH: T + (tci + 1) * TCH]
                for ft in (2, 0, 3, 1):  # k_p0, q_p0, k_p1, q_p1
                    pqk = ps.tile([128, TCH], F32, tag="pa", name=f"pqk_{tci}_{ft}")
                    for cc in range(NCC):
                        nc.tensor.matmul(
                            pqk[:],
                            wqk_sb[:, cc * 512 + ft * 128: cc * 512 + (ft + 1) * 128],
                            xt[:, cc * TCH:(cc + 1) * TCH],
                            start=(cc == 0), stop=(cc == NCC - 1))
                    Xc = qk_dst[ft][:]
                    if has_battn:
                        nc.scalar.activation(Xc, pqk[:], AF.Identity,
                                             bias=bqk_sb[:, ft:ft + 1])
                    else:
                        nc.vector.tensor_copy(Xc, pqk[:])
                    # RoPE immediately (swap halves via SBUF->SBUF DMAs)
                    X = qk_dst[ft]
                    xs = rsb.tile([128, TCH], MMDT, tag="xswap", name=f"xs_{tci}_{ft}")
                    nc.gpsimd.dma_start(out=xs[0:32, :], in_=X[32:64, :])
                    nc.gpsimd.dma_start(out=xs[32:64, :], in_=X[0:32, :])
                    nc.gpsimd.dma_start(out=xs[64:96, :], in_=X[96:128, :])
                    nc.sync.dma_start(out=xs[96:128, :], in_=X[64:96, :])
                    nc.vector.tensor_tensor(xs[:], xs[:], sinc, ALU.mult)
                    nc.vector.tensor_tensor(Xc, Xc, cosc, ALU.mult)
                    nc.vector.tensor_tensor(Xc, Xc, xs[:], ALU.add)
                # v: two j-pairs, each one PSUM bank
                for jj in range(2):
                    pv = ps.tile([128, 2 * HPC * D], F32, tag="pa", name=f"pv_{tci}_{jj}")
                    for j2 in range(2):
                        j = jj * 2 + j2
                        for cc in range(NCC):
                            nc.tensor.matmul(
                                pv[:, j2 * 256:(j2 + 1) * 256],
                                xt[:, cc * TCH + j * 128: cc * TCH + (j + 1) * 128],
                                wv_sb[:, cc * 256:(cc + 1) * 256],
                                start=(cc == 0), stop=(cc == NCC - 1))
                    for j2 in range(2):
                        tt = tci * 4 + jj * 2 + j2
                        dst = vaug[tt][:, 0:HPC * (D + 1)].rearrange(
                            "p (h e) -> p h e", e=D + 1)[:, :, 0:D]
                        src = pv[:, j2 * 256:(j2 + 1) * 256].rearrange(
                            "p (h e) -> p h e", e=D)
                        if has_battn:
                            nc.vector.scalar_tensor_tensor(
                                dst, src, 0.0,
                                vbias_sb[:].rearrange("p (h e) -> p h e", e=D),
                                ALU.add, ALU.add)
                        else:
                            nc.vector.tensor_copy(dst, src)


            def emit_scores(p, qc, ktp):
                """scores + exp + mask for one ktile-pair; returns et tiles."""
                sc = [ps.tile([128, 2 * TCH], F32, tag="sc",
                              name=f"sc_{p}_{qc}_{ktp}_{h}") for h in range(2)]
                et = [esb.tile([128, 2 * TCH], MMDT, tag="et",
                               name=f"et_{p}_{qc}_{ktp}_{h}") for h in range(2)]
                for half in range(2):
                    kt = 2 * ktp + half
                    for h in range(2):
                        nc.tensor.matmul(
                            sc[h][:, half * TCH:(half + 1) * TCH],
                            kTc[p][kt // 4][h * 64:(h + 1) * 64,
                                            (kt % 4) * 128:(kt % 4 + 1) * 128],
                            qTc[p][qc][h * 64:(h + 1) * 64, :],
                            start=True, stop=True,
                            tile_position=(64 * h, 0))
                for h in range(2):
                    nc.scalar.activation(et[h][:], sc[h][:], AF.Exp, scale=0.125)
                if _DEBUG and p == 0 and qc == 0 and ktp == 0:
                    dt_ = osb.tile([128, 2 * TCH], F32, tag="dbge")
                    nc.vector.tensor_copy(dt_[:], et[0][:])
                    nc.sync.dma_start(out=dbg_et[:], in_=dt_[:])
                for half in range(2):
                    kt = 2 * ktp + half
                    m = kt - 4 * qc
                    if m >= 0:
                        w = 128 * (m + 1)
                        off = half * TCH
                        for h in range(2):
                            nc.vector.tensor_tensor(
                                et[h][:, off:off + w], et[h][:, off:off + w],
                                mask_sb[:, m * TCH: m * TCH + w], ALU.mult)
                return et

            def emit_av(p, qc, ktp, et, yps):
                nk = 4 * qc + 4
                for half in range(2):
                    kt = 2 * ktp + half
                    for h in range(2):
                        hh = 2 * p + h
                        nc.tensor.matmul(
                            yps[h][:],
                            vaug[kt][:, hh * (D + 1):(hh + 1) * (D + 1)],
                            et[h][:, half * TCH:(half + 1) * TCH],
                            start=(kt == 0), stop=(kt == nk - 1))

            def emit_norm(p, qc, yps):
                dln = nsb.tile([1, 2 * TCH], F32, tag="dln", name=f"dln_{p}_{qc}")
                for h in range(2):
                    nc.scalar.activation(dln[0:1, h * TCH:(h + 1) * TCH],
                                         yps[h][D:D + 1, :], AF.Ln)
                drec = nsb.tile([1, 2 * TCH], MMDT, tag="drec",
                                name=f"drec_{p}_{qc}")
                nc.scalar.activation(drec[:], dln[:], AF.Exp, scale=-1.0)
                pb = ps.tile([128, TCH], F32, tag="pa", name=f"pb_{p}_{qc}")
                for h in range(2):
                    nc.tensor.matmul(pb[h * 64:(h + 1) * 64, :], ones_sb[:],
                                     drec[0:1, h * TCH:(h + 1) * TCH],
                                     start=True, stop=True,
                                     tile_position=(0, 64 * h))
                rb = nsb.tile([128, TCH], MMDT, tag="rb", name=f"rb_{p}_{qc}")
                nc.vector.tensor_copy(rb[:], pb[:])
                if _DEBUG and p == 0 and qc == 0:
                    dr_ = osb.tile([1, 2 * TCH], F32, tag="dbgr")
                    nc.vector.tensor_copy(dr_[:], dln[:])
                    nc.sync.dma_start(out=dbg_dr[:], in_=dr_[:])
                for h in range(2):
                    nc.vector.tensor_tensor(
                        yTc[p][qc][h * 64:(h + 1) * 64, :],
                        yps[h][0:D, :], rb[h * 64:(h + 1) * 64, :], ALU.mult)

            def emit_b(qc):
                nk2 = 2 * qc + 2
                yps0 = [ps.tile([D + 1, TCH], F32, tag="yacc",
                                name=f"yps_0_{qc}_{h}") for h in range(2)]
                for ktp in range(nk2):
                    et = emit_scores(0, qc, ktp)
                    emit_av(0, qc, ktp, et, yps0)
                # prefetch p1 scores for 2 ktile-pairs before norm(p0) so the
                # PE has work while the p0 reciprocal chain runs
                npre = min(2, nk2)
                pre = [emit_scores(1, qc, ktp) for ktp in range(npre)]
                emit_norm(0, qc, yps0)
                yps1 = [ps.tile([D + 1, TCH], F32, tag="yacc",
                                name=f"yps_1_{qc}_{h}") for h in range(2)]
                for ktp in range(nk2):
                    et = pre[ktp] if ktp < npre else emit_scores(1, qc, ktp)
                    emit_av(1, qc, ktp, et, yps1)
                emit_norm(1, qc, yps1)

            # ---- software pipeline over t-chunks ----
            emit_a(0)
            for tci in range(NTC):
                if tci + 1 < NTC:
                    emit_a(tci + 1)
                if tci > 0:
                    _emit_c(nc, ps, osb, yTc, wp_sb, out_d, tci - 1, False)
                emit_b(tci)

            # final q-chunk's output projection
            _emit_c(nc, ps, osb, yTc, wp_sb, out_d, NTC - 1, True)

            if _DEBUG:
                for srcs, dst_t in [(qTc[0], dbg_q0), (kTc[0], dbg_k0),
                                    (yTc[0], dbg_y0)]:
                    for c in range(NTC):
                        dt_ = osb.tile([128, TCH], F32, tag="dbg")
                        nc.vector.tensor_copy(dt_[:], srcs[c][:])
                        nc.sync.dma_start(
                            out=dst_t[:, c * TCH:(c + 1) * TCH], in_=dt_[:])
                dv_ = osb.tile([128, HPC * (D + 1)], F32, tag="dbgv")
                nc.vector.tensor_copy(dv_[:], vaug[0][:])
                nc.sync.dma_start(out=dbg_va[:], in_=dv_[:])

    nc.finalize()
    return nc


def _rope_tables():
    dd = (np.arange(128) % 64) % 32
    fraction = (2.0 * np.arange(32, dtype=np.float32) / 64).astype(np.float32)
    timescale = (np.float32(10000.0) ** fraction).astype(np.float32)
    pos = np.arange(T, dtype=np.float32)
    ang = (pos[None, :] / timescale[dd][:, None]).astype(np.float32)  # [128, T]
    cos_t = np.cos(ang).astype(np.float32)
    sin_t = np.sin(ang).astype(np.float32)
    sgn = np.where((np.arange(128) % 64) < 32, np.float32(-1.0), np.float32(1.0))
    sin_signed = (sin_t * sgn[:, None]).astype(np.float32)
    return cos_t, sin_signed


def _mask_tiles():
    masks = np.zeros((128, 4 * TCH), np.float32)
    r = np.arange(128)[:, None]
    c = np.arange(TCH)[None, :]
    for m in range(4):
        masks[:, m * TCH:(m + 1) * TCH] = (c >= 128 * m + r).astype(np.float32)
    return masks


def kernel(x, W_attn, b_attn, W_proj, b_proj):
    x = np.asarray(x, np.float32)
    W_attn = np.asarray(W_attn, np.float32)
    b_attn = np.asarray(b_attn, np.float32)
    W_proj = np.asarray(W_proj, np.float32)
    b_proj = np.asarray(b_proj, np.float32)

    _patch_act_tables()
    has_battn = bool(np.any(b_attn != 0))
    key = ("v5", has_battn, _DEBUG)
    if key not in _prog_cache:
        _prog_cache[key] = _build_program(has_battn)
    nc = _prog_cache[key]

    import ml_dtypes
    bf = ml_dtypes.bfloat16
    cos_t, sin_signed = _rope_tables()
    trig = np.concatenate([cos_t, sin_signed], axis=1).astype(bf)  # [128, 4096]
    masks = _mask_tiles().astype(bf)
    hsel = np.ones((2, 128), bf)

    in_maps = []
    for core in range(NCORES):
        b, g = divmod(core, HPC)
        hs = [HPC * g + i for i in range(HPC)]
        qkcols = []
        for base in (0, C):  # q tiles then k tiles
            for p in range(2):
                for i in (2 * p, 2 * p + 1):
                    qkcols += [base + hs[i] * D + d for d in range(D)]
        vcols = [2 * C + h * D + d for h in hs for d in range(D)]
        rows = [h * D + d for h in hs for d in range(D)]

        wqk = np.ascontiguousarray(
            W_attn[:, qkcols].reshape(NCC, 128, 512).transpose(1, 0, 2)
            .reshape(128, NCC * 512)).astype(bf)
        wv = np.ascontiguousarray(
            W_attn[:, vcols].reshape(NCC, 128, 256).transpose(1, 0, 2)
            .reshape(128, NCC * 256)).astype(bf)
        wp = np.ascontiguousarray(
            W_proj[rows, :].reshape(2, 128, C).transpose(1, 0, 2)
            .reshape(128, 2 * C)).astype(bf)
        xt = np.ascontiguousarray(
            x[b].reshape(NTC, TCH, NCC, 128).transpose(3, 0, 2, 1)
            .reshape(128, NTC * NCC * TCH)).astype(bf)

        im = {
            "xt": xt, "wqk": wqk, "wv": wv, "wp": wp,
            "trig": trig, "masks": masks, "hsel": hsel,
        }
        if has_battn:
            im["bqk"] = np.ascontiguousarray(
                b_attn[qkcols].reshape(4, 128).T).astype(np.float32)
            im["vbias"] = np.tile(b_attn[vcols], (128, 1)).astype(bf)
        in_maps.append(im)

    trace = bool(os.environ.get("TRNK_TRACE"))
    if trace:
        try:
            import ntff_shim  # noqa: F401
        except ImportError:
            trace = False
    res = run_bass_kernel_spmd(nc, in_maps, list(range(NCORES)), trace=trace,
                               tmpdir=os.environ.get("TRNK_TMPDIR") or None)
    if trace:
        globals()["_last_exec_time_ns"] = res.exec_time_ns
        globals()["_last_result"] = res

    globals()["_dbg_results"] = res.results
    out = np.zeros((B, T, C), np.float32)
    for core in range(NCORES):
        b = core // HPC
        out[b] += np.asarray(res.results[core]["out"], np.float32)
    out += b_proj[None, None, :]
    return out
